# revision 1
# baseline (speedup 1.0000x reference)
"""Trainium2 Bass kernel for nn_AttentionBlock (B=2,T=2048,C=1024,H=16,D=64,F=4096).

Sharding: 8 cores = 2 batches x 4 query-chunks of 512 tokens. Each core
computes K/V for the full sequence of its batch locally (no collectives)
and attention + MLP for its own 512 query tokens. The host rolls the
token axis per core so each core's own tokens sit at columns 0..511 of
the full-T buffers -- one SPMD program serves all cores, and softmax /
attention sums are key-order invariant so the roll needs no undoing.
All matmuls run in bf16 with f32 PSUM accumulation; the residual stream,
softmax denominators and rms statistics stay in f32.

Softmax: scores stay in PSUM; exp() runs on the scalar engine straight
from PSUM, and the alibi bias is folded in as a bf16 multiply by
host-precomputed exp(alibi) at the vector engine's 2x 16-bit rate.

All DRAM-resident operands are host-relaid so every DMA reads
per-partition contiguous lines >= 1KB.

Self-contained: hardcodes shapes; host-side prep slices/casts/transposes
inputs per core, device output is [C, TQ] per core, host reassembles.
"""
import os
from contextlib import ExitStack

import numpy as np
import ml_dtypes

BF16 = ml_dtypes.bfloat16

B, T, C, H, D, F = 2, 2048, 1024, 16, 64, 4096
NCORES, G = 8, 4
TQ = T // G                 # 512 query tokens per core
HD = H * D                  # 1024
EPS = 1e-8
CC = C // 128               # 8 channel chunks
TC = T // 128               # 16 token chunks
FC = F // 128               # 32 hidden chunks
NV = TQ // 128              # 4 own V chunks
VW = 4 * 65                 # padded V row width (64 dims + denom col per head)
HL = 4                      # heads per core

# stream_shuffle permutes lanes within each 32-partition quadrant
# (out[s*32+i] = in[s*32+mask[i]]). We host-permute the head dim so rope
# partners (d, d+32) sit on adjacent partitions; the swap is then [1,0,3,2,..].
ROPE_PERM = np.arange(64).reshape(2, 32).T.reshape(-1)   # [0,32,1,33,...]
SWAP_MASK = [i ^ 1 for i in range(32)]

LAST_RESULTS = None  # BassKernelResults of the last run (for test.py)


def host_prep(inputs):
    x = np.asarray(inputs["x"], np.float32)
    alibi = np.asarray(inputs["alibi"], np.float32)
    rot = np.asarray(inputs["rotational"], np.float32)
    g_att = np.asarray(inputs["g_att"], np.float32)
    g_mlp = np.asarray(inputs["g_mlp"], np.float32)
    w_qkv = np.asarray(inputs["w_qkv"], np.float32)
    w_att_out = np.asarray(inputs["w_att_out"], np.float32)
    w_mlp_in = np.asarray(inputs["w_mlp_in"], np.float32)
    b_mlp_in = np.asarray(inputs["b_mlp_in"], np.float32)
    w_mlp_out = np.asarray(inputs["w_mlp_out"], np.float32)
    b_mlp_out = np.asarray(inputs["b_mlp_out"], np.float32)

    ea = np.exp(alibi)                            # folded in post-exp multiply

    wg = w_qkv * g_att[None, :]                   # fold g_att into qkv
    wg[:HD] *= 1.0 / np.sqrt(np.float32(D))       # fold attn scale into w_q

    # permute each head's 64 q/k output dims so rope pairs are adjacent
    wqk_p = wg[:2 * HD].reshape(2, H, 64, C)[:, :, ROPE_PERM, :].reshape(
        2 * HD, C)
    wqkvT = np.ascontiguousarray(
        np.concatenate([wqk_p, wg[2 * HD:]], 0).T).astype(BF16)    # [C, 3HD]
    # q/k slabs: [16, 128, CC*128], slab[e][p, c*128+m] = wqkvT[c*128+p, e*128+m]
    wqkT_t = np.ascontiguousarray(
        wqkvT[:, :2 * HD].reshape(CC, 128, 16, 128).transpose(2, 1, 0, 3)
        .reshape(16, 128, CC * 128))
    wvT_r = np.ascontiguousarray(
        wqkvT[:, 2 * HD:].reshape(CC, 128, HD))                    # [8,128,HD]
    woT = np.ascontiguousarray(w_att_out.T).astype(BF16)           # [HD, C]
    w_inT = np.ascontiguousarray((w_mlp_in * g_mlp[None, :]).T).astype(BF16)
    w_inT_t = np.ascontiguousarray(
        w_inT.reshape(CC, 128, FC, 128).transpose(2, 1, 0, 3)
        .reshape(FC, 128, CC * 128))                               # [32,128,1024]
    w_outT = np.ascontiguousarray(w_mlp_out.T).astype(BF16)        # [F, C]
    w_outT_t = np.ascontiguousarray(
        w_outT.reshape(FC, 128, 2, 4, 128).transpose(0, 2, 1, 3, 4)
        .reshape(FC, 2, 128, 512))                                 # [f,half,128,512]

    b_in_t = np.ascontiguousarray(b_mlp_in.reshape(FC, 128).T)     # [128, 32]
    b_out_t = np.ascontiguousarray(b_mlp_out.reshape(CC, 128).T)   # [128, 8]

    cosT = np.cos(rot).T.astype(np.float32)                        # [D, T]
    sinT = np.sin(rot).T.astype(np.float32)
    sgn = np.where(np.arange(D) < D // 2, -1.0, 1.0).astype(np.float32)
    ssinT = sinT * sgn[:, None]
    cosT = cosT[ROPE_PERM]                             # match head-dim perm
    # pre-swap the sign-sin rows: device computes swap(x * ss) == swap(x) * s
    ssinT = ssinT[ROPE_PERM][np.arange(64) ^ 1]
    cs2T = np.ascontiguousarray(np.tile(cosT, (2, 1))).astype(BF16)  # [128, T]
    ss2T = np.ascontiguousarray(np.tile(ssinT, (2, 1))).astype(BF16)

    xT = {b: np.ascontiguousarray(x[b].T) for b in range(B)}       # [C,T] f32
    return dict(ea=ea, wqkT_t=wqkT_t, wvT_r=wvT_r, woT_full=woT,
                w_inT_t=w_inT_t, w_outT_t=w_outT_t, b_in_t=b_in_t,
                b_out_t=b_out_t, cs2T=cs2T, ss2T=ss2T, xT=xT)


def core_inputs(hp, core):
    b, j = core // G, core % G
    q0 = j * TQ
    # 4 own heads (4j..4j+3); queries/keys stay in global token order.
    # ea_t[hl, qc, p, tc*TQ+q] = exp(alibi[4j+hl, qc*TQ+q, tc*128+p])
    ea_t = np.ascontiguousarray(
        hp["ea"][4 * j:4 * j + 4]
        .reshape(4, G, TQ, TC, 128)
        .transpose(0, 1, 4, 3, 2).reshape(4, G, 128, TC * TQ)).astype(BF16)
    # q/k slabs for own heads: q-dim chunks {2j, 2j+1}, k-dim {8+2j, 8+2j+1}
    wqk4 = np.ascontiguousarray(
        hp["wqkT_t"][[2 * j, 2 * j + 1, CC + 2 * j, CC + 2 * j + 1]])
    # v columns for own heads (256 wide) per c-chunk
    wv4 = np.ascontiguousarray(hp["wvT_r"][:, :, 256 * j:256 * (j + 1)])
    # out-proj slabs vs own 256 head-dims: wo4[cc][p, hl, m]
    #   = w_att_out[cc*128+m, j*256 + hl*128 + p]
    wo4 = np.ascontiguousarray(
        hp["woT_full"][256 * j:256 * (j + 1)]
        .reshape(2, 128, CC, 128).transpose(2, 1, 0, 3)
        .reshape(CC, 128, 2 * 128))
    return dict(
        xT_own=np.ascontiguousarray(hp["xT"][b][:, q0:q0 + TQ]),
        xbf=hp["xT"][b].astype(BF16),
        ea_t=ea_t,
        cs2T=hp["cs2T"], ss2T=hp["ss2T"],
        wqkT_t=wqk4, wvT_r=wv4, woT_t=wo4,
        w_inT_t=hp["w_inT_t"], w_outT_t=hp["w_outT_t"],
        b_in_t=hp["b_in_t"], b_out_t=hp["b_out_t"],
    )


def build(nc, tc, io, ctx, phases="all"):
    import concourse.bass as bass
    import concourse.mybir as mybir
    from concourse.bass import ts
    from concourse.masks import make_identity

    dt = mybir.dt
    AF = mybir.ActivationFunctionType
    OP = mybir.AluOpType
    f32, bf16 = dt.float32, dt.bfloat16

    def pool(name, bufs, space="SBUF"):
        return ctx.enter_context(tc.tile_pool(name=name, bufs=bufs, space=space))

    consts = pool("consts", 1)
    ones_col = consts.tile([128, 1], bf16, tag="ones", name="ones")
    nc.vector.memset(ones_col[:, :], 1.0)
    ones_f32 = consts.tile([128, 1], f32, tag="ones32", name="ones32")
    nc.vector.memset(ones_f32[:, :], 1.0)
    b_in_sb = consts.tile([128, FC], f32, tag="b_in", name="b_in")
    nc.sync.dma_start(b_in_sb[:, :], io["b_in_t"][:, :])
    b_out_sb = consts.tile([128, CC], f32, tag="b_out", name="b_out")
    nc.sync.dma_start(b_out_sb[:, :], io["b_out_t"][:, :])

    y1_pool = pool("y1", CC)
    y2_pool = pool("y2", CC)

    qkv_scope = ExitStack()

    def qpool(name, bufs, space="SBUF"):
        return qkv_scope.enter_context(
            tc.tile_pool(name=name, bufs=bufs, space=space))

    QT_pool = qpool("QT", 2)
    KT_pool = qpool("KT", 2)
    V_pool = qpool("V", TC)
    QT, KT, V = [], [], []

    with ExitStack() as p1:
        def ppool(name, bufs, space="SBUF"):
            return p1.enter_context(
                tc.tile_pool(name=name, bufs=bufs, space=space))

        # x (full rolled sequence, bf16) [C, T] as CC tiles of [128, T]
        xbf_pool = ppool("xbf", CC)
        xbf = []
        for c in range(CC):
            t = xbf_pool.tile([128, T], bf16, tag="xbf", name="xbf")
            nc.sync.dma_start(t[:, :], io["xbf"][ts(c, 128), :])
            xbf.append(t)

        csr_pool = ppool("csr", 1)
        cs_r = csr_pool.tile([128, T], bf16, tag="csr", name="csr")
        ss_r = csr_pool.tile([128, T], bf16, tag="ssr", name="ssr")
        r_col = csr_pool.tile([128, TC], f32, tag="rcol", name="rcol")

        stats_scope = ExitStack()
        rms1_pool = stats_scope.enter_context(tc.tile_pool(name="rms1", bufs=1))
        sq_pool = stats_scope.enter_context(tc.tile_pool(name="sq", bufs=2))
        rms1_ps = stats_scope.enter_context(
            tc.tile_pool(name="rms1ps", bufs=1, space="PSUM"))

        # full-T rmsnorm stats from bf16 x; squares split across the two
        # engines that are idle during the ramp (ACT + Pool), keeping DVE
        # free for the rope chain
        ssq = [rms1_ps.tile([1, 512], f32, tag=f"ssq{i}", name=f"ssq{i}")
               for i in range(4)]
        for c in range(CC):
            sq = sq_pool.tile([128, T], bf16, tag="sq", name="sq")
            if c % 2 == 0:
                nc.scalar.activation(sq[:, :], xbf[c][:, :], AF.Square)
            else:
                nc.gpsimd.tensor_tensor(sq[:, :], xbf[c][:, :],
                                        xbf[c][:, :], OP.mult)
            for n in range(4):
                nc.tensor.matmul(ssq[n][:, :], ones_col[:, :],
                                 sq[:, ts(n, 512)],
                                 start=(c == 0), stop=(c == CC - 1))
        r_sb = rms1_pool.tile([1, T], f32, tag="r1", name="r1")
        r128 = rms1_pool.tile([128, T], f32, tag="r1b", name="r1b")
        for n in range(4):
            nc.scalar.activation(r128[0:1, ts(n, 512)], ssq[n][:, :],
                                 AF.Sqrt, bias=0.0, scale=1.0 / C)
        nc.vector.reciprocal(r_sb[:, :], r128[0:1, :])
        nc.gpsimd.partition_broadcast(r128[:, :], r_sb[:, :])

        # rope tables with r folded in
        with tc.tile_pool(name="cstmp", bufs=1) as cst:
            for dst, src_name in ((cs_r, "cs2T"), (ss_r, "ss2T")):
                tmp = cst.tile(list(dst.shape), bf16, tag="cstmp", name="cstmp")
                nc.sync.dma_start(tmp[:, :], io[src_name][:, :])
                nc.vector.tensor_tensor(dst[:, :], tmp[:, :], r128[:, :],
                                        OP.mult)

        # transpose r -> per-token-chunk columns for the V scaling
        rt_ps = stats_scope.enter_context(
            tc.tile_pool(name="rtps", bufs=2, space="PSUM"))
        for t in range(TC):
            ps = rt_ps.tile([128, 1], f32, tag="rtps", name="rtps")
            nc.tensor.matmul(ps[:, :], r_sb[0:1, ts(t, 128)],
                             ones_f32[0:1, 0:1], start=True, stop=True)
            nc.vector.tensor_copy(r_col[:, t:t + 1], ps[:, :])
        stats_scope.close()

        # ---------------- QKV projections + rope -----------------------
        wslab_pool = ppool("wslab", 3)
        qk_ps = ppool("qkps", 3, "PSUM")
        rope_pool = ppool("ropet", 6)

        def rope(dst_ap, ps, cs_ap, ss_ap):
            qc = rope_pool.tile([128, 512], bf16, tag="ropeA", name="ropeA")
            nc.vector.tensor_tensor(qc[:, :], ps[:, :], cs_ap, OP.mult)
            tmp = rope_pool.tile([128, 512], bf16, tag="ropeB", name="ropeB")
            nc.vector.tensor_tensor(tmp[:, :], ps[:, :], ss_ap, OP.mult)
            qs = rope_pool.tile([128, 512], bf16, tag="ropeC", name="ropeC")
            nc.vector.stream_shuffle(qs[:, :], tmp[:, :], SWAP_MASK)
            # final add on Pool: SBUF-only operands, and Pool idles here
            nc.gpsimd.tensor_tensor(dst_ap, qc[:, :], qs[:, :], OP.add)

        for e in range(2):       # K for own 4 heads (full T), stays in SBUF
            slab = wslab_pool.tile([128, CC, 128], bf16, tag="wslab",
                                   name="wslab")
            nc.sync.dma_start(
                slab[:, :, :],
                io["wqkT_t"][2 + e].rearrange("p (c m) -> p c m", m=128))
            kt = KT_pool.tile([128, T], bf16, tag="KT", name="KT")
            for n in range(4):
                ps = qk_ps.tile([128, 512], f32, tag="qkps", name="qkps")
                for c in range(CC):
                    nc.tensor.matmul(ps[:, :], slab[:, c, :],
                                     xbf[c][:, ts(n, 512)],
                                     start=(c == 0), stop=(c == CC - 1))
                rope(kt[:, ts(n, 512)], ps, cs_r[:, ts(n, 512)],
                     ss_r[:, ts(n, 512)])
            KT.append(kt)

        for e in range(2):       # Q for own 4 heads (full T)
            slab = wslab_pool.tile([128, CC, 128], bf16, tag="wslab",
                                   name="wslab")
            nc.sync.dma_start(
                slab[:, :, :],
                io["wqkT_t"][e].rearrange("p (c m) -> p c m", m=128))
            qt = QT_pool.tile([128, T], bf16, tag="QT", name="QT")
            for n in range(4):
                ps = qk_ps.tile([128, 512], f32, tag="qkps", name="qkps")
                for c in range(CC):
                    nc.tensor.matmul(ps[:, :], slab[:, c, :],
                                     xbf[c][:, ts(n, 512)],
                                     start=(c == 0), stop=(c == CC - 1))
                rope(qt[:, ts(n, 512)], ps, cs_r[:, ts(n, 512)],
                     ss_r[:, ts(n, 512)])
            QT.append(qt)

        # V for own 4 heads (full T), padded per-head layout
        wv_pool = ppool("wvr", CC)
        wv = []
        for c in range(CC):
            t = wv_pool.tile([128, 256], bf16, tag="wvr", name="wvr")
            nc.sync.dma_start(t[:, :], io["wvT_r"][c][:, :])
            wv.append(t)
        for tcn in range(TC):
            vt = V_pool.tile([128, VW], bf16, tag="V", name="V")
            vt3 = vt.rearrange("p (h x) -> p h x", x=65)
            nc.vector.memset(vt3[:, :, 64:65], 1.0)
            ps = qk_ps.tile([128, 512], f32, tag="qkps", name="qkps")
            for c in range(CC):
                nc.tensor.matmul(ps[:, 0:256], xbf[c][:, ts(tcn, 128)],
                                 wv[c][:, :],
                                 start=(c == 0), stop=(c == CC - 1))
            nc.vector.tensor_scalar_mul(
                vt3[:, :, 0:64],
                ps[:, 0:256].rearrange("p (h d) -> p h d", d=64),
                r_col[:, tcn:tcn + 1])
            V.append(vt)

    al_scope = ExitStack()
    al_pool = al_scope.enter_context(tc.tile_pool(name="alibi", bufs=4))

    def load_alibi(hl, qc):
        al = al_pool.tile([128, TC * TQ], bf16, tag="alibi", name="alibi")
        nc.sync.dma_start(al[:, :], io["ea_t"][hl, qc][:, :])
        return al

    al_cache = {k: load_alibi(*k) for k in ((0, 0), (1, 0), (0, 1))}

    if phases == "qkv":
        qkv_scope.close()
        al_scope.close()
        return

    # ---------------- attention ---------------------------------------
    yatt_scope = ExitStack()
    yatt_pool = yatt_scope.enter_context(tc.tile_pool(name="yatt", bufs=2))
    yatt = [yatt_pool.tile([128, T], bf16, tag="yatt", name="yatt")
            for _ in range(2)]

    with tc.tile_pool(name="scps", bufs=3, space="PSUM") as sc_ps, \
         tc.tile_pool(name="avps", bufs=2, space="PSUM") as av_ps, \
         tc.tile_pool(name="expt", bufs=6) as e_pool, \
         tc.tile_pool(name="attm", bufs=6) as m_pool, \
         tc.tile_pool(name="attsm", bufs=2) as sm_pool:

        for hp in range(2):
            hl0, hl1 = 2 * hp, 2 * hp + 1
            for qc in range(G):
                als = []
                for hl in (hl0, hl1):
                    al = al_cache.pop((hl, qc), None)
                    if al is None:
                        al = load_alibi(hl, qc)
                    als.append(al)
                av0 = av_ps.tile([65, 512], f32, tag="av", name="av0")
                av1 = av_ps.tile([65, 512], f32, tag="av", name="av1")
                for g in range(8):
                    ps0 = sc_ps.tile([128, 1024], f32, tag="scps",
                                     name="scps0")
                    ps1 = sc_ps.tile([128, 1024], f32, tag="scps",
                                     name="scps1")
                    for s in range(2):
                        tkc = 2 * g + s
                        nc.tensor.matmul(ps0[:, ts(s, 512)],
                                         KT[hp][0:64, ts(tkc, 128)],
                                         QT[hp][0:64, ts(qc, 512)],
                                         start=True, stop=True,
                                         tile_position=(0, 0))
                        nc.tensor.matmul(ps1[:, ts(s, 512)],
                                         KT[hp][64:128, ts(tkc, 128)],
                                         QT[hp][64:128, ts(qc, 512)],
                                         start=True, stop=True,
                                         tile_position=(64, 0))
                    # exp on ACT straight from PSUM, then bf16 alibi multiply
                    e0 = e_pool.tile([128, 1024], bf16, tag="expt",
                                     name="expt0")
                    nc.scalar.activation(e0[:, :], ps0[:, :], AF.Exp)
                    e1 = e_pool.tile([128, 1024], bf16, tag="expt",
                                     name="expt1")
                    nc.scalar.activation(e1[:, :], ps1[:, :], AF.Exp)
                    m0 = m_pool.tile([128, 1024], bf16, tag="attm",
                                     name="attm0")
                    nc.vector.tensor_tensor(m0[:, :], e0[:, :],
                                            als[0][:, ts(g, 1024)], OP.mult)
                    m1 = m_pool.tile([128, 1024], bf16, tag="attm",
                                     name="attm1")
                    nc.vector.tensor_tensor(m1[:, :], e1[:, :],
                                            als[1][:, ts(g, 1024)], OP.mult)
                    for s in range(2):
                        tkc = 2 * g + s
                        nc.tensor.matmul(av0[:, :],
                                         V[tkc][:, hl0 * 65:hl0 * 65 + 65],
                                         m0[:, ts(s, 512)],
                                         start=(tkc == 0),
                                         stop=(tkc == TC - 1))
                        nc.tensor.matmul(av1[:, :],
                                         V[tkc][:, hl1 * 65:hl1 * 65 + 65],
                                         m1[:, ts(s, 512)],
                                         start=(tkc == 0),
                                         stop=(tkc == TC - 1))
                for idx, av in ((0, av0), (1, av1)):
                    rr = sm_pool.tile([1, 512], f32, tag="rr", name="rr")
                    nc.vector.reciprocal(rr[:, :], av[64:65, :])
                    r64 = sm_pool.tile([64, 512], f32, tag="r64", name="r64")
                    nc.gpsimd.partition_broadcast(r64[:, :], rr[:, :])
                    nc.vector.tensor_tensor(
                        yatt[hp][idx * 64:(idx + 1) * 64, ts(qc, 512)],
                        av[0:64, :], r64[:, :], OP.mult)

    # ------- partial out proj (own 256 head dims, all T) + ReduceScatter
    rs_pool = ExitStack()
    rs_dram = rs_pool.enter_context(
        tc.tile_pool(name="rsdram", bufs=1, space="DRAM"))
    f8 = dt.float8e4
    rs_in = rs_dram.tile([G * C * TQ], f8, tag="rsin", name="rsin")
    rs_out = rs_dram.tile([C * TQ], f8, tag="rsout", name="rsout")
    with tc.tile_pool(name="woslab", bufs=2) as wo_pool, \
         tc.tile_pool(name="y1p", bufs=3) as y1p_pool, \
         tc.tile_pool(name="aops", bufs=2, space="PSUM") as ao_ps:
        for cc in range(CC):
            slab = wo_pool.tile([128, 2, 128], bf16, tag="woslab",
                                name="woslab")
            nc.sync.dma_start(
                slab[:, :, :],
                io["woT_t"][cc].rearrange("p (hd m) -> p hd m", m=128))
            y1p = y1p_pool.tile([128, T], f8, tag="y1p", name="y1p")
            for nh in range(2):     # 1024-wide PSUM halves: 2 copies per cc
                ps = ao_ps.tile([128, 1024], f32, tag="aops", name="aops")
                for s in range(2):
                    for hd in range(2):
                        nc.tensor.matmul(ps[:, ts(s, 512)], slab[:, hd, :],
                                         yatt[hd][:, ts(2 * nh + s, 512)],
                                         start=(hd == 0), stop=(hd == 1))
                if nh == 0:
                    nc.vector.tensor_copy(y1p[:, ts(nh, 1024)], ps[:, :])
                else:
                    nc.scalar.activation(y1p[:, ts(nh, 1024)], ps[:, :],
                                         AF.Copy)
            for w in range(G):
                nc.sync.dma_start(
                    rs_in[w * C * TQ + cc * 128 * TQ:
                          w * C * TQ + (cc + 1) * 128 * TQ].rearrange(
                        "(p q) -> p q", q=TQ),
                    y1p[:, ts(w, 512)])
    nc.gpsimd.collective_compute(
        "ReduceScatter", mybir.AluOpType.add,
        replica_groups=[[0, 1, 2, 3], [4, 5, 6, 7]],
        ins=[rs_in[:].opt()], outs=[rs_out[:].opt()])

    yatt_scope.close()
    al_scope.close()
    qkv_scope.close()   # free QT/KT/V

    # prefetch the first MLP input slabs while the ReduceScatter runs --
    # issued before the rs_out reads so they don't queue behind the
    # collective-blocked DMAs
    mlp_scope = ExitStack()
    wi_pool = mlp_scope.enter_context(tc.tile_pool(name="wislab", bufs=6))
    wi_pre = []
    for f in range(6):
        slab = wi_pool.tile([128, CC, 128], bf16, tag="wislab", name="wislab")
        nc.sync.dma_start(
            slab[:, :, :],
            io["w_inT_t"][f].rearrange("p (c m) -> p c m", m=128))
        wi_pre.append(slab)

    # own-token y1 = reduce-scattered partial + residual
    y1 = []
    xo_scope = ExitStack()
    xo_pool = xo_scope.enter_context(tc.tile_pool(name="xo", bufs=CC))
    xo = []
    for c in range(CC):
        t = xo_pool.tile([128, TQ], f32, tag="xo", name="xo")
        nc.sync.dma_start(t[:, :], io["xT_own"][ts(c, 128), :])
        xo.append(t)
    with tc.tile_pool(name="rsb", bufs=3) as rsb_pool:
        for c in range(CC):
            rsb = rsb_pool.tile([128, TQ], f8, tag="rsb", name="rsb")
            nc.sync.dma_start(
                rsb[:, :],
                rs_out[c * 128 * TQ:(c + 1) * 128 * TQ].rearrange(
                    "(p q) -> p q", q=TQ))
            t = y1_pool.tile([128, TQ], f32, tag="y1", name="y1")
            nc.vector.tensor_tensor(t[:, :], rsb[:, :], xo[c][:, :], OP.add)
            y1.append(t)
    xo_scope.close()
    rs_pool.close()

    if phases.startswith("att"):
        return

    # ---------------- rmsnorm #2 ---------------------------------------
    y2 = []
    with tc.tile_pool(name="rms2", bufs=4) as rms2_pool, \
         tc.tile_pool(name="rms2ps", bufs=1, space="PSUM") as rms2_ps:
        ssq2 = rms2_ps.tile([1, 512], f32, tag="ssq2", name="ssq2")
        for c in range(CC):
            sq2 = rms2_pool.tile([128, TQ], bf16, tag="sq2", name="sq2")
            nc.scalar.activation(sq2[:, :], y1[c][:, :], AF.Square)
            nc.tensor.matmul(ssq2[:, :], ones_col[:, :], sq2[:, :],
                             start=(c == 0), stop=(c == CC - 1))
        r2 = rms2_pool.tile([1, TQ], f32, tag="r2", name="r2")
        sd2 = rms2_pool.tile([1, TQ], f32, tag="sd2", name="sd2")
        nc.scalar.activation(sd2[:, :], ssq2[:, :], AF.Sqrt,
                             bias=0.0, scale=1.0 / C)
        nc.vector.reciprocal(r2[:, :], sd2[:, :])
        r2128 = rms2_pool.tile([128, TQ], f32, tag="r2b", name="r2b")
        nc.gpsimd.partition_broadcast(r2128[:, :], r2[:, :])
        for c in range(CC):
            t = y2_pool.tile([128, TQ], bf16, tag="y2", name="y2")
            if c % 2 == 0:
                nc.vector.tensor_tensor(t[:, :], y1[c][:, :], r2128[:, :],
                                        OP.mult)
            else:
                nc.gpsimd.tensor_tensor(t[:, :], y1[c][:, :], r2128[:, :],
                                        OP.mult)
            y2.append(t)

    # ---------------- MLP ----------------------------------------------
    with tc.tile_pool(name="hT", bufs=FC) as h_pool, \
         tc.tile_pool(name="woslab2", bufs=3) as wo2_pool, \
         tc.tile_pool(name="mlpips", bufs=2, space="PSUM") as mi_ps, \
         tc.tile_pool(name="mlpops", bufs=4, space="PSUM") as mo_ps, \
         tc.tile_pool(name="mlpfin", bufs=4) as fin_pool:

        hT = []
        for half in range(2):
            ops = [mo_ps.tile([128, TQ], f32, tag="mops", name="mops")
                   for _ in range(4)]
            for f in range(FC):
                if half == 0:
                    if f < len(wi_pre):
                        slab = wi_pre[f]
                    else:
                        slab = wi_pool.tile([128, CC, 128], bf16,
                                            tag="wislab", name="wislab")
                        nc.sync.dma_start(
                            slab[:, :, :],
                            io["w_inT_t"][f].rearrange("p (c m) -> p c m",
                                                       m=128))
                    ip = mi_ps.tile([128, TQ], f32, tag="mips", name="mips")
                    for c in range(CC):
                        nc.tensor.matmul(ip[:, :], slab[:, c, :], y2[c][:, :],
                                         start=(c == 0), stop=(c == CC - 1))
                    hf = h_pool.tile([128, TQ], bf16, tag="hT", name="hT")
                    nc.scalar.activation(hf[:, :], ip[:, :], AF.Gelu,
                                         bias=b_in_sb[:, f:f + 1])
                    hT.append(hf)
                oslab = wo2_pool.tile([128, 4, 128], bf16, tag="woslab2",
                                      name="woslab2")
                nc.sync.dma_start(
                    oslab[:, :, :],
                    io["w_outT_t"][f, half].rearrange("p (i m) -> p i m",
                                                      m=128))
                for i in range(4):
                    nc.tensor.matmul(ops[i][:, :], oslab[:, i, :], hT[f][:, :],
                                     start=(f == 0), stop=(f == FC - 1))
            for i in range(4):
                cc = 4 * half + i
                tmp = fin_pool.tile([128, TQ], f32, tag="fin", name="fin")
                nc.vector.tensor_tensor(tmp[:, :], ops[i][:, :], y1[cc][:, :],
                                        OP.add)
                out_sb = fin_pool.tile([128, TQ], f32, tag="fin2", name="fin2")
                # bias add alternates DVE/Pool so the tail chain isn't
                # serial on one engine
                if i % 2 == 0:
                    nc.vector.tensor_scalar_add(out_sb[:, :], tmp[:, :],
                                                b_out_sb[:, cc:cc + 1])
                else:
                    nc.gpsimd.tensor_scalar_add(out_sb[:, :], tmp[:, :],
                                                b_out_sb[:, cc:cc + 1])
                nc.sync.dma_start(io["outT"][ts(cc, 128), :], out_sb[:, :])
    mlp_scope.close()


_NC_CACHE = {}


def _build_nc(repeats=1, phases="all"):
    key = (repeats, phases)
    if key in _NC_CACHE:
        return _NC_CACHE[key]
    from contextlib import ExitStack as _ES
    from concourse import bacc
    import concourse.tile as tile
    import concourse.mybir as mybir

    dt = mybir.dt
    nc = bacc.Bacc("TRN2", target_bir_lowering=False, debug=False,
                   num_devices=NCORES)

    io = {}
    spec = dict(
        xT_own=((C, TQ), dt.float32),
        xbf=((C, T), dt.bfloat16),
        ea_t=((HL, G, 128, TC * TQ), dt.bfloat16),
        cs2T=((128, T), dt.bfloat16), ss2T=((128, T), dt.bfloat16),
        wqkT_t=((4, 128, CC * 128), dt.bfloat16),
        wvT_r=((CC, 128, 256), dt.bfloat16),
        woT_t=((CC, 128, 256), dt.bfloat16),
        w_inT_t=((FC, 128, CC * 128), dt.bfloat16),
        w_outT_t=((FC, 2, 128, 512), dt.bfloat16),
        b_in_t=((128, FC), dt.float32), b_out_t=((128, CC), dt.float32),
    )
    for name, (shape, d) in spec.items():
        io[name] = nc.dram_tensor(name, list(shape), d, kind="ExternalInput").ap()
    io["outT"] = nc.dram_tensor("outT", [C, TQ], dt.float32,
                                kind="ExternalOutput").ap()

    with tile.TileContext(nc, pool_alloc_mode="queue") as tc:
        for rep in range(repeats):
            if rep:
                tc.strict_bb_all_engine_barrier()
            with ExitStack() as ctx:
                build(nc, tc, io, ctx, phases=phases)
    nc.compile()
    _NC_CACHE[key] = nc
    return nc


def _run(in_maps, trace):
    global LAST_RESULTS
    import concourse.bass_utils as bass_utils
    nc = _build_nc()
    results = bass_utils.run_bass_kernel_spmd(
        nc, in_maps, core_ids=list(range(NCORES)), trace=trace)
    LAST_RESULTS = results
    return results


def bench(in_maps, iters=3, reps=4, phases="all"):
    """Marginal per-kernel device time: build two NEFFs (1x body, `iters`x
    body with all-engine barriers between repeats), run both with
    device-resident inputs, report (T_iters - T_1)/(iters-1)."""
    import time
    import jax
    from jax.sharding import Mesh, PartitionSpec
    from jax.experimental.shard_map import shard_map
    import concourse.mybir as mybir
    from concourse import bass2jax
    from concourse.bass2jax import _bass_exec_p, install_neuronx_cc_hook

    install_neuronx_cc_hook()
    timings = {}
    for n_rep in (1, iters):
        nc = _build_nc(n_rep, phases)
        in_names, out_names, out_avals, zero_outs = [], [], [], []
        partition_name = (nc.partition_id_tensor.name
                          if nc.partition_id_tensor else None)
        for alloc in nc.m.functions[0].allocations:
            if not isinstance(alloc, mybir.MemoryLocationSet):
                continue
            name = alloc.memorylocations[0].name
            if alloc.kind == "ExternalInput":
                if name != partition_name:
                    in_names.append(name)
            elif alloc.kind == "ExternalOutput":
                shape = tuple(alloc.tensor_shape)
                dtype = mybir.dt.np(alloc.dtype)
                out_names.append(name)
                out_avals.append(jax.core.ShapedArray(shape, dtype))
                zero_outs.append(np.zeros(shape, dtype))
        n_params = len(in_names)
        n_outs = len(out_avals)
        all_in_names = list(in_names) + list(out_names)
        if partition_name is not None:
            all_in_names.append(partition_name)

        def _body(*args, _nc=nc, _avals=tuple(out_avals),
                  _innames=tuple(all_in_names), _outnames=tuple(out_names),
                  _pname=partition_name):
            operands = list(args)
            if _pname is not None:
                operands.append(bass2jax.partition_id_tensor())
            outs = _bass_exec_p.bind(
                *operands, out_avals=_avals, in_names=_innames,
                out_names=_outnames, lowering_input_output_aliases=(),
                sim_require_finite=True, sim_require_nnan=True, nc=_nc)
            return tuple(outs)

        devices = jax.devices()[:NCORES]
        mesh = Mesh(np.asarray(devices), ("core",))
        in_specs = (PartitionSpec("core"),) * (n_params + n_outs)
        out_specs = (PartitionSpec("core"),) * n_outs
        per_core = [[np.asarray(m[name]) for name in in_names]
                    for m in in_maps]
        concat_in = [np.concatenate([per_core[c][i] for c in range(NCORES)],
                                    axis=0) for i in range(n_params)]

        donate = tuple(range(n_params, n_params + n_outs))
        fn = jax.jit(shard_map(_body, mesh=mesh, in_specs=in_specs,
                               out_specs=out_specs, check_rep=False),
                     donate_argnums=donate, keep_unused=True)
        samples = []
        for i in range(reps + 1):
            # fresh, value-varying zero buffers each call defeat any
            # result-memoization in the execution path
            zs = [np.full((NCORES * z.shape[0], *z.shape[1:]), 1e-6 * i,
                          z.dtype) for z in zero_outs]
            t0 = time.perf_counter()
            outs = fn(*concat_in, *zs)
            jax.block_until_ready(outs)
            dt = time.perf_counter() - t0
            if i > 0:
                samples.append(dt)
        timings[n_rep] = min(samples)
    per_iter = (timings[iters] - timings[1]) / (iters - 1)
    return per_iter * 1e9, timings[1] * 1e9


def kernel(**inputs):
    hp = host_prep(inputs)
    in_maps = []
    for core in range(NCORES):
        ci = core_inputs(hp, core)
        in_maps.append({k: np.ascontiguousarray(v) for k, v in ci.items()})
    trace = bool(int(os.environ.get("KERNEL_TRACE", "0")))
    results = _run(in_maps, trace)
    out = np.zeros((B, T, C), np.float32)
    for core in range(NCORES):
        b, j = core // G, core % G
        out[b, j * TQ:(j + 1) * TQ, :] = results.results[core]["outT"].T
    return out


if __name__ == "__main__":
    import reference
    inputs = reference.setup_inputs()
    out = kernel(**{k: np.asarray(v) for k, v in inputs.items()})
    exp = np.asarray(reference.reference(**inputs))
    err = np.abs(out - exp).max() / np.abs(exp).max()
    print("rel(absmax) err:", err)

